# revision 16
# baseline (speedup 1.0000x reference)
"""Region-wise 15x15 conv + 16x16 box-sum on 8 Trainium2 cores.

Math (per b,c):
  corr[i,j] = sum_{u,v} padded_img[i+u, j+v] * kernels[(i//16)*16 + j//16, u, v]
              for i,j in [0,241)   (padded = 7-zero-pad of 256x256 image)
  out[y,x]  = sum_{i in [y-15,y], j in [x-15,x]} corr[i,j]    (truncated box)

Sharding: core k owns region-rows {2k, 2k+1} -> corr rows i in [32k, 32k+32).
Each core emits a [12bc, 47, 256] output slab (rows y in [32k, 32k+47));
host overlap-adds slabs.

On-core compute:
  corrT[j, bc, i] = sum_u matmul( ToepW_u[x, j], T_img[x, (bc, i+u)] )
    - ToepW (host-built, banded Toeplitz of kernel row u along x) is the
      PE-stationary operand; x-chunks [0,128),[114,242),[228,271) map to
      j-chunks [0,114),[114,228),[228,241); 15 u-taps accumulate in PSUM.
  box-sum along i: log2 shift-adds (1,2,4,8) on DVE over the free dim.
  box-sum along x: matmul with banded-ones rhs -> out[(bc,y), x] in PSUM.
"""

import os
import sys

import numpy as np

sys.path.insert(0, "/opt/trn_rl_repo/concourse")

KS = 15  # kernel size
R = 16  # region size
P = 7  # pad
B, C, H, W = 4, 3, 256, 256
BC = B * C  # 12
NCORES = 8
HO = H - R + 1  # 241 sliding positions per dim
XP = H + 2 * P  # 270 padded extent; x index range [0, 271) with 1 slack
NR = 47  # padded rows per core (i in [0,32) plus u in [0,15))
NI = 62  # padded i-axis: i_pad = i_local + 15, i_local in [-15, 47)
NY = 47  # output rows per core slab
JC = [(0, 114, 0), (114, 114, 114), (228, 13, 228)]  # (j0, jlen, x0) chunks
XL = [128, 128, 43]  # x-chunk partition counts

_CACHE = {}


def _build_core_inputs(image, kernels):
    """Host-side marshaling: per-core transposed image slabs, Toeplitz
    weight tiles, band-ones tiles, and validity masks."""
    img = image.reshape(BC, H, W)
    ker = kernels.reshape(H // R * (W // R), KS, KS)  # [256, 15, 15]
    in_maps = []
    for k in range(NCORES):
        # T_img[x, bc, r]: padded row (32k + r), padded col x ( = img col x-7)
        timg = np.zeros((XP + 1, BC, NR), np.float32)
        r0 = 32 * k - P
        lo = max(0, r0)
        hi = min(H, r0 + NR)
        timg[P : P + W, :, lo - r0 : hi - r0] = img[:, lo:hi, :].transpose(2, 0, 1)

        # Toeplitz weights (partition-first): toep01[xl, ri, u, c, jl] =
        #   ker[kidx, u, v] with v = (x0_c + xl) - (j0_c + jl) in [0,15),
        #   kidx = rg*16 + jg//16
        toep01 = np.zeros((128, 2, 15, 2, 114), np.float32)
        toep2 = np.zeros((43, 2, 15, 13), np.float32)
        for ri in range(2):
            rg = 2 * k + ri
            for c in range(3):
                j0, jlen, x0 = JC[c]
                xl = np.arange(XL[c])[:, None]
                jl = np.arange(jlen)[None, :]
                v = (x0 + xl) - (j0 + jl)
                band = (v >= 0) & (v < KS)
                vv = np.clip(v, 0, KS - 1)
                kk = np.broadcast_to(rg * 16 + (j0 + jl) // 16, v.shape)
                for u in range(15):
                    vals_u = np.where(band, ker[kk, u, vv], 0.0)
                    if c < 2:
                        toep01[: XL[c], ri, u, c, :jlen] = vals_u
                    else:
                        toep2[: XL[c], ri, u, :jlen] = vals_u

        # band-ones for the x-direction box sum: band[jl, x] = 1 if 0 <= x - jg <= 15
        band01 = np.zeros((128, 2, 256), np.float32)
        band2 = np.zeros((13, 256), np.float32)
        xs = np.arange(256)[None, :]
        for c in range(3):
            j0, jlen, _ = JC[c]
            jg = j0 + np.arange(jlen)[:, None]
            bb = ((xs - jg >= 0) & (xs - jg <= KS)).astype(np.float32)
            if c < 2:
                band01[:jlen, c, :] = bb
            else:
                band2[:jlen, :] = bb

        # validity mask over (ri, bc, i): i_global = (2k+ri)*16 + i < 241
        mask = np.zeros((128, 2, BC, 16), np.float32)
        for ri in range(2):
            ig = (2 * k + ri) * 16 + np.arange(16)
            mask[:, ri, :, :] = (ig < HO).astype(np.float32)[None, None, :]

        in_maps.append(
            {
                "timg": timg,
                "toep01": np.ascontiguousarray(toep01),
                "toep2": np.ascontiguousarray(toep2),
                "band01": np.ascontiguousarray(band01),
                "band2": np.ascontiguousarray(band2),
                "mask": np.ascontiguousarray(mask),
            }
        )
    return in_maps


def _build_bass():
    """Raw bass (explicit semaphores, <=1 wait per instruction — this
    walrus build rejects multi-wait instructions)."""
    import contextlib

    import concourse.bass as bass
    import concourse.mybir as mybir

    f32 = mybir.dt.float32
    nc = bass.Bass()
    d_timg = nc.dram_tensor("timg", (XP + 1, BC, NR), f32, kind="ExternalInput")
    d_toep01 = nc.dram_tensor("toep01", (128, 2, 15, 2, 114), f32, kind="ExternalInput")
    d_toep2 = nc.dram_tensor("toep2", (43, 2, 15, 13), f32, kind="ExternalInput")
    d_band01 = nc.dram_tensor("band01", (128, 2, 256), f32, kind="ExternalInput")
    d_band2 = nc.dram_tensor("band2", (13, 256), f32, kind="ExternalInput")
    d_mask = nc.dram_tensor("mask", (128, 2, BC, 16), f32, kind="ExternalInput")
    d_out = nc.dram_tensor("out", (BC * NY, 256), f32, kind="ExternalOutput")

    with contextlib.ExitStack() as st:
        ec = st.enter_context
        # SBUF
        timg = [ec(nc.sbuf_tensor(f"s_timg{c}", [XL[c], BC * NR], f32)) for c in range(3)]
        toep01 = ec(nc.sbuf_tensor("s_toep01", [128, 2 * 15 * 2 * 114], f32))
        toep2 = ec(nc.sbuf_tensor("s_toep2", [43, 2 * 15 * 13], f32))
        band01 = ec(nc.sbuf_tensor("s_band01", [128, 2 * 256], f32))
        band2 = ec(nc.sbuf_tensor("s_band2", [13, 256], f32))
        mask = ec(nc.sbuf_tensor("s_mask", [128, 2 * BC * 16], f32))
        corrT = [ec(nc.sbuf_tensor(f"s_corrT{c}", [128, BC * NI], f32)) for c in range(3)]
        sa = [ec(nc.sbuf_tensor(f"s_sa{c}", [128, BC * NI], f32)) for c in range(3)]
        sb = [ec(nc.sbuf_tensor(f"s_sb{c}", [128, BC * NI], f32)) for c in range(3)]
        sbf = [ec(nc.sbuf_tensor(f"s_sbf{c}", [128, BC * NY], f32)) for c in range(3)]
        outbuf = ec(nc.sbuf_tensor("s_outbuf", [128, 6 * 256], f32))
        # PSUM: full-bank tensors to guarantee bank exclusivity
        ps = [ec(nc.psum_tensor(f"p_ps{g}", [128, 512], f32)) for g in range(6)]
        pb = [ec(nc.psum_tensor(f"p_pb{g}", [128, 512], f32)) for g in range(2)]
        # semaphores
        s_in = ec(nc.semaphore(name="s_in"))
        s_pe = ec(nc.semaphore(name="s_pe"))
        s_dve = ec(nc.semaphore(name="s_dve"))
        s_pb = ec(nc.semaphore(name="s_pb"))
        s_cp = ec(nc.semaphore(name="s_cp"))
        s_out = ec(nc.semaphore(name="s_out"))
        block = ec(nc.Block())

        def toep_ap(c, ri, u):
            jlen = JC[c][1]
            if c < 2:
                base = ((ri * 15 + u) * 2 + c) * 114
                return toep01[:, base : base + jlen]
            base = (ri * 15 + u) * 13
            return toep2[:, base : base + jlen]

        def timg_ap(c, ri, u):
            v = timg[c][:].rearrange("p (a b) -> p a b", a=BC)
            return v[:, :, 16 * ri + u : 16 * ri + u + 16]

        @block.sync
        def _(sync):
            sync.dma_start(toep01[:], d_toep01[:].rearrange("p a b c j -> p (a b c j)")).then_inc(s_in, 16)
            sync.dma_start(toep2[:], d_toep2[:].rearrange("p a b j -> p (a b j)")).then_inc(s_in, 16)
            for c in range(3):
                x0 = JC[c][2] if c > 0 else 0
                sync.dma_start(
                    timg[c][:],
                    d_timg[x0 : x0 + XL[c]].rearrange("p a b -> p (a b)"),
                ).then_inc(s_in, 16)
            sync.dma_start(band01[:], d_band01[:].rearrange("p a b -> p (a b)")).then_inc(s_in, 16)
            sync.dma_start(band2[:], d_band2[:]).then_inc(s_in, 16)
            sync.dma_start(mask[:], d_mask[:].rearrange("p a b c -> p (a b c)")).then_inc(s_in, 16)
            for m in range(6):
                sync.wait_ge(s_cp, m + 1)
                sync.dma_start(
                    d_out[2 * NY * m : 2 * NY * (m + 1)],
                    outbuf[: 2 * NY, 256 * m : 256 * (m + 1)],
                ).then_inc(s_out, 16)

        @block.tensor
        def _(tensor):
            tensor.wait_ge(s_in, 16 * 8)
            for ri in range(2):
                for c in range(3):
                    jlen = JC[c][1]
                    g = ri * 3 + c
                    pview = ps[g][:jlen, : BC * 16].rearrange("p (a b) -> p a b", a=BC)
                    for u in range(15):
                        mm = nc.tensor.matmul(
                            pview,
                            toep_ap(c, ri, u),
                            timg_ap(c, ri, u),
                            start=(u == 0),
                            stop=(u == 14),
                        )
                        if u == 14:
                            mm.then_inc(s_pe, 1)
            tensor.wait_ge(s_dve, 3)
            for m in range(6):
                if m >= 2:
                    tensor.wait_ge(s_cp, m - 1)
                for c in range(3):
                    jlen = JC[c][1]
                    rhs = (
                        band01[:jlen, 256 * c : 256 * (c + 1)]
                        if c < 2
                        else band2[:jlen, :]
                    )
                    mm = nc.tensor.matmul(
                        pb[m % 2][: 2 * NY, :256],
                        sbf[c][:jlen, 2 * NY * m : 2 * NY * (m + 1)],
                        rhs,
                        start=(c == 0),
                        stop=(c == 2),
                    )
                    if c == 2:
                        mm.then_inc(s_pb, 1)

        @block.vector
        def _(vector):
            for c in range(3):
                nc.vector.memset(corrT[c][:], 0.0)
            for ri in range(2):
                for c in range(3):
                    jlen = JC[c][1]
                    g = ri * 3 + c
                    vector.wait_ge(s_pe, g + 1)
                    dst = corrT[c][:jlen].rearrange("p (a b) -> p a b", a=BC)[
                        :, :, 15 + 16 * ri : 31 + 16 * ri
                    ]
                    mview = mask[:jlen].rearrange("p (r a b) -> p r a b", r=2, a=BC)
                    pview = ps[g][:jlen, : BC * 16].rearrange("p (a b) -> p a b", a=BC)
                    nc.vector.tensor_mul(dst, pview, mview[:, ri])
            for c in range(3):
                jlen = JC[c][1]
                cv = corrT[c][:jlen].rearrange("p (a b) -> p a b", a=BC)
                av = sa[c][:jlen].rearrange("p (a b) -> p a b", a=BC)
                bv = sb[c][:jlen].rearrange("p (a b) -> p a b", a=BC)
                fv = sbf[c][:jlen].rearrange("p (a b) -> p a b", a=BC)
                ln = NI - 1  # 61
                nc.vector.tensor_add(av[:, :, :ln], cv[:, :, :ln], cv[:, :, 1 : 1 + ln])
                ln -= 2  # 59
                nc.vector.tensor_add(bv[:, :, :ln], av[:, :, :ln], av[:, :, 2 : 2 + ln])
                ln -= 4  # 55
                nc.vector.tensor_add(av[:, :, :ln], bv[:, :, :ln], bv[:, :, 4 : 4 + ln])
                ln -= 8  # 47
                nc.vector.tensor_add(
                    fv[:, :, :ln], av[:, :, :ln], av[:, :, 8 : 8 + ln]
                ).then_inc(s_dve, 1)
            for m in range(6):
                vector.wait_ge(s_pb, m + 1)
                nc.vector.tensor_copy(
                    outbuf[: 2 * NY, 256 * m : 256 * (m + 1)],
                    pb[m % 2][: 2 * NY, :256],
                ).then_inc(s_cp, 1)

    return nc


def _build_bass_tile_unused():
    import concourse.bass as bass
    import concourse.mybir as mybir
    from concourse import tile

    f32 = mybir.dt.float32
    nc = bass.Bass()
    d_timg = nc.dram_tensor("timg", (XP + 1, BC, NR), f32, kind="ExternalInput")
    d_toep01 = nc.dram_tensor("toep01", (128, 2, 15, 2, 114), f32, kind="ExternalInput")
    d_toep2 = nc.dram_tensor("toep2", (43, 2, 15, 13), f32, kind="ExternalInput")
    d_band01 = nc.dram_tensor("band01", (128, 2, 256), f32, kind="ExternalInput")
    d_band2 = nc.dram_tensor("band2", (13, 256), f32, kind="ExternalInput")
    d_mask = nc.dram_tensor("mask", (128, 2, BC, 16), f32, kind="ExternalInput")
    d_out = nc.dram_tensor("out", (BC * NY, 256), f32, kind="ExternalOutput")

    with tile.TileContext(nc) as tc:
        with (
            tc.tile_pool(name="const", bufs=1) as cpool,
            tc.tile_pool(name="work", bufs=1) as wpool,
            tc.tile_pool(name="psum", bufs=2, space=bass.MemorySpace.PSUM) as ppool,
            tc.tile_pool(name="psum_o", bufs=2, space=bass.MemorySpace.PSUM) as opool,
        ):
            # ---- stage in constants ----
            timg = [cpool.tile([XL[c], BC, NR], f32, tag=f"timg{c}", name=f"timg{c}") for c in range(3)]
            for c in range(3):
                x0 = JC[c][2] if c > 0 else 0
                nc.gpsimd.dma_start(timg[c][:], d_timg[x0 : x0 + XL[c]])
            toep01 = cpool.tile([128, 2, 15, 2, 114], f32)
            nc.gpsimd.dma_start(toep01[:], d_toep01[:])
            toep2 = cpool.tile([43, 2, 15, 13], f32)
            nc.gpsimd.dma_start(toep2[:], d_toep2[:])
            band01 = cpool.tile([128, 2, 256], f32)
            nc.gpsimd.dma_start(band01[:], d_band01[:])
            band2 = cpool.tile([13, 256], f32)
            nc.gpsimd.dma_start(band2[:], d_band2[:])
            mask = cpool.tile([128, 2, BC, 16], f32)
            nc.gpsimd.dma_start(mask[:], d_mask[:])

            # corrT[c][jl, bc, i_pad], i_pad = 15 + 16*ri + i
            corrT = [wpool.tile([128, BC, NI], f32, tag=f"corrT{c}", name=f"corrT{c}") for c in range(3)]
            for c in range(3):
                nc.vector.memset(corrT[c][:], 0.0)

            # ---- main matmuls: corr via 15 accumulated taps ----
            for ri in range(2):
                ps = [ppool.tile([128, BC, 16], f32, tag=f"ps{c}", name=f"ps{c}") for c in range(3)]
                for u in range(15):
                    for c in range(3):
                        jlen = JC[c][1]
                        lhsT = (
                            toep01[:, ri, u, c, :jlen]
                            if c < 2
                            else toep2[:, ri, u, :jlen]
                        )
                        rhs = timg[c][:, :, 16 * ri + u : 16 * ri + u + 16]
                        nc.tensor.matmul(
                            ps[c][:jlen],
                            lhsT,
                            rhs,
                            start=(u == 0),
                            stop=(u == 14),
                        )
                for c in range(3):
                    jlen = JC[c][1]
                    nc.vector.tensor_mul(
                        corrT[c][:jlen, :, 15 + 16 * ri : 31 + 16 * ri],
                        ps[c][:jlen],
                        mask[:jlen, ri],
                    )

            # ---- box-sum along i (free dim): shifts 1,2,4,8 ----
            sa = [wpool.tile([128, BC, NI], f32, tag=f"sa{c}", name=f"sa{c}") for c in range(3)]
            sb = [wpool.tile([128, BC, NI], f32, tag=f"sb{c}", name=f"sb{c}") for c in range(3)]
            sbf = [wpool.tile([128, BC * NY], f32, tag=f"sbf{c}", name=f"sbf{c}") for c in range(3)]
            for c in range(3):
                jlen = JC[c][1]
                ln = NI - 1  # 61
                nc.vector.tensor_add(
                    sa[c][:jlen, :, :ln],
                    corrT[c][:jlen, :, :ln],
                    corrT[c][:jlen, :, 1 : 1 + ln],
                )
                ln -= 2  # 59
                nc.vector.tensor_add(
                    sb[c][:jlen, :, :ln],
                    sa[c][:jlen, :, :ln],
                    sa[c][:jlen, :, 2 : 2 + ln],
                )
                ln -= 4  # 55
                nc.vector.tensor_add(
                    sa[c][:jlen, :, :ln],
                    sb[c][:jlen, :, :ln],
                    sb[c][:jlen, :, 4 : 4 + ln],
                )
                ln -= 8  # 47
                # final step writes a compact [j, bc*47] tile so stage-B's
                # stationary operand has a single free dim
                sbv = sbf[c][:jlen].rearrange("p (a b) -> p a b", a=BC)
                nc.vector.tensor_add(
                    sbv[:, :, :ln],
                    sa[c][:jlen, :, :ln],
                    sa[c][:jlen, :, 8 : 8 + ln],
                )

            # ---- box-sum along x via band-ones matmul; out[(bc,y), x] ----
            outbuf = wpool.tile([128, 6, 256], f32)
            for m in range(6):
                ob = opool.tile([128, 256], f32, tag="ob", name="ob")
                for c in range(3):
                    jlen = JC[c][1]
                    lhsT = sbf[c][:jlen, 2 * NY * m : 2 * NY * (m + 1)]
                    rhs = band01[:jlen, c] if c < 2 else band2[:jlen]
                    nc.tensor.matmul(
                        ob[: 2 * NY],
                        lhsT,
                        rhs,
                        start=(c == 0),
                        stop=(c == 2),
                    )
                nc.vector.tensor_copy(outbuf[: 2 * NY, m], ob[: 2 * NY])
            for m in range(6):
                nc.sync.dma_start(
                    d_out[2 * NY * m : 2 * NY * (m + 1)], outbuf[: 2 * NY, m]
                )
    return nc


def kernel(image, kernels):
    image = np.ascontiguousarray(np.asarray(image, np.float32))
    kernels = np.ascontiguousarray(np.asarray(kernels, np.float32))
    from concourse import bass_utils

    if "nc" not in _CACHE:
        _CACHE["nc"] = _build_bass()
    nc = _CACHE["nc"]
    in_maps = _build_core_inputs(image, kernels)
    import time as _time

    trace = bool(int(os.environ.get("KTRACE", "0")))
    try:
        t0 = _time.time()
        res = bass_utils.run_bass_kernel_spmd(
            nc, in_maps, core_ids=list(range(NCORES)), trace=trace
        )
        dt = _time.time() - t0
    except ModuleNotFoundError:
        # axon NTFF profiling hook unavailable in this container
        t0 = _time.time()
        res = bass_utils.run_bass_kernel_spmd(
            nc, in_maps, core_ids=list(range(NCORES)), trace=False
        )
        dt = _time.time() - t0
    if res.exec_time_ns is not None:
        print(f"HW exec time: {res.exec_time_ns} ns")
    else:
        # no profiler available: report end-to-end device dispatch wall
        # (upper bound on HW exec; includes PJRT transfer + launch)
        print(f"HW exec time: {int(dt * 1e9)} ns (wall upper bound, no NTFF hook)")
    out = np.zeros((BC, H, W), np.float32)
    for k in range(NCORES):
        slab = res.results[k]["out"].reshape(BC, NY, 256)
        y0 = 32 * k
        y1 = min(H, y0 + NY)
        out[:, y0:y1, :] += slab[:, : y1 - y0, :]
    return out.reshape(B, C, H, W)


# revision 18
# speedup vs baseline: 1.3020x; 1.3020x over previous
"""Region-wise 15x15 conv + 16x16 box-sum on 8 Trainium2 cores.

Math (per b,c):
  corr[i,j] = sum_{u,v} padded_img[i+u, j+v] * kernels[(i//16)*16 + j//16, u, v]
              for i,j in [0,241)   (padded = 7-zero-pad of 256x256 image)
  out[y,x]  = sum_{i in [y-15,y], j in [x-15,x]} corr[i,j]    (truncated box)

Sharding: core k owns region-rows {2k, 2k+1} -> corr rows i in [32k, 32k+32).
Each core emits a [12bc, 47, 256] output slab (rows y in [32k, 32k+47));
host overlap-adds slabs.

On-core compute:
  corrT[j, bc, i] = sum_u matmul( ToepW_u[x, j], T_img[x, (bc, i+u)] )
    - ToepW (host-built, banded Toeplitz of kernel row u along x) is the
      PE-stationary operand; x-chunks [0,128),[114,242),[228,271) map to
      j-chunks [0,114),[114,228),[228,241); 15 u-taps accumulate in PSUM.
  box-sum along i: log2 shift-adds (1,2,4,8) on DVE over the free dim.
  box-sum along x: matmul with banded-ones rhs -> out[(bc,y), x] in PSUM.
"""

import os
import sys

import numpy as np

sys.path.insert(0, "/opt/trn_rl_repo/concourse")

KS = 15  # kernel size
R = 16  # region size
P = 7  # pad
B, C, H, W = 4, 3, 256, 256
BC = B * C  # 12
NCORES = 8
HO = H - R + 1  # 241 sliding positions per dim
XP = H + 2 * P  # 270 padded extent; x index range [0, 271) with 1 slack
NR = 47  # padded rows per core (i in [0,32) plus u in [0,15))
NI = 62  # padded i-axis: i_pad = i_local + 15, i_local in [-15, 47)
NY = 47  # output rows per core slab
JC = [(0, 114, 0), (114, 114, 114), (228, 13, 228)]  # (j0, jlen, x0) chunks
XL = [128, 128, 43]  # x-chunk partition counts

_CACHE = {}


def _build_core_inputs(image, kernels):
    """Host-side marshaling: per-core transposed image slabs, Toeplitz
    weight tiles, band-ones tiles, and validity masks."""
    img = image.reshape(BC, H, W)
    ker = kernels.reshape(H // R * (W // R), KS, KS)  # [256, 15, 15]
    in_maps = []
    for k in range(NCORES):
        # T_img[x, bc, r]: padded row (32k + r), padded col x ( = img col x-7)
        timg = np.zeros((XP + 1, BC, NR), np.float32)
        r0 = 32 * k - P
        lo = max(0, r0)
        hi = min(H, r0 + NR)
        timg[P : P + W, :, lo - r0 : hi - r0] = img[:, lo:hi, :].transpose(2, 0, 1)

        # Toeplitz weights (partition-first): toep01[xl, ri, u, c, jl] =
        #   ker[kidx, u, v] with v = (x0_c + xl) - (j0_c + jl) in [0,15),
        #   kidx = rg*16 + jg//16
        toep01 = np.zeros((128, 2, 15, 2, 114), np.float32)
        toep2 = np.zeros((43, 2, 15, 13), np.float32)
        for ri in range(2):
            rg = 2 * k + ri
            for c in range(3):
                j0, jlen, x0 = JC[c]
                xl = np.arange(XL[c])[:, None]
                jl = np.arange(jlen)[None, :]
                v = (x0 + xl) - (j0 + jl)
                band = (v >= 0) & (v < KS)
                vv = np.clip(v, 0, KS - 1)
                kk = np.broadcast_to(rg * 16 + (j0 + jl) // 16, v.shape)
                for u in range(15):
                    vals_u = np.where(band, ker[kk, u, vv], 0.0)
                    if c < 2:
                        toep01[: XL[c], ri, u, c, :jlen] = vals_u
                    else:
                        toep2[: XL[c], ri, u, :jlen] = vals_u

        # band-ones for the x-direction box sum: band[jl, x] = 1 if 0 <= x - jg <= 15
        band01 = np.zeros((128, 2, 256), np.float32)
        band2 = np.zeros((13, 256), np.float32)
        xs = np.arange(256)[None, :]
        for c in range(3):
            j0, jlen, _ = JC[c]
            jg = j0 + np.arange(jlen)[:, None]
            bb = ((xs - jg >= 0) & (xs - jg <= KS)).astype(np.float32)
            if c < 2:
                band01[:jlen, c, :] = bb
            else:
                band2[:jlen, :] = bb

        # validity mask over (ri, bc, i): i_global = (2k+ri)*16 + i < 241
        mask = np.zeros((128, 2, BC, 16), np.float32)
        for ri in range(2):
            ig = (2 * k + ri) * 16 + np.arange(16)
            mask[:, ri, :, :] = (ig < HO).astype(np.float32)[None, None, :]

        import ml_dtypes

        bf = ml_dtypes.bfloat16
        in_maps.append(
            {
                "timg": timg.astype(bf),
                "toep01": np.ascontiguousarray(toep01).astype(bf),
                "toep2": np.ascontiguousarray(toep2).astype(bf),
                "band01": np.ascontiguousarray(band01),
                "band2": np.ascontiguousarray(band2),
                "mask": np.ascontiguousarray(mask),
            }
        )
    return in_maps


def _build_bass():
    """Raw bass (explicit semaphores, <=1 wait per instruction — this
    walrus build rejects multi-wait instructions)."""
    import contextlib

    import concourse.bass as bass
    import concourse.mybir as mybir

    f32 = mybir.dt.float32
    bf16 = mybir.dt.bfloat16
    nc = bass.Bass()
    d_timg = nc.dram_tensor("timg", (XP + 1, BC, NR), bf16, kind="ExternalInput")
    d_toep01 = nc.dram_tensor("toep01", (128, 2, 15, 2, 114), bf16, kind="ExternalInput")
    d_toep2 = nc.dram_tensor("toep2", (43, 2, 15, 13), bf16, kind="ExternalInput")
    d_band01 = nc.dram_tensor("band01", (128, 2, 256), f32, kind="ExternalInput")
    d_band2 = nc.dram_tensor("band2", (13, 256), f32, kind="ExternalInput")
    d_mask = nc.dram_tensor("mask", (128, 2, BC, 16), f32, kind="ExternalInput")
    d_out = nc.dram_tensor("out", (BC * NY, 256), f32, kind="ExternalOutput")

    with contextlib.ExitStack() as st:
        ec = st.enter_context
        # SBUF
        timg = [ec(nc.sbuf_tensor(f"s_timg{c}", [XL[c], BC * NR], bf16)) for c in range(3)]
        toep01 = ec(nc.sbuf_tensor("s_toep01", [128, 2 * 15 * 2 * 114], bf16))
        toep2 = ec(nc.sbuf_tensor("s_toep2", [43, 2 * 15 * 13], bf16))
        band01 = ec(nc.sbuf_tensor("s_band01", [128, 2 * 256], f32))
        band2 = ec(nc.sbuf_tensor("s_band2", [13, 256], f32))
        mask = ec(nc.sbuf_tensor("s_mask", [128, 2 * BC * 16], f32))
        corrT = [ec(nc.sbuf_tensor(f"s_corrT{c}", [128, BC * NI], f32)) for c in range(3)]
        sa = [ec(nc.sbuf_tensor(f"s_sa{c}", [128, BC * NI], f32)) for c in range(3)]
        sb = [ec(nc.sbuf_tensor(f"s_sb{c}", [128, BC * NI], f32)) for c in range(3)]
        sbf = [ec(nc.sbuf_tensor(f"s_sbf{c}", [128, BC * NY], f32)) for c in range(3)]
        outbuf = ec(nc.sbuf_tensor("s_outbuf", [128, 6 * 256], f32))
        # PSUM: full-bank tensors to guarantee bank exclusivity
        ps = [ec(nc.psum_tensor(f"p_ps{g}", [128, 512], f32)) for g in range(6)]
        pb = [ec(nc.psum_tensor(f"p_pb{g}", [128, 512], f32)) for g in range(2)]
        # semaphores
        s_in = ec(nc.semaphore(name="s_in"))
        s_pe = ec(nc.semaphore(name="s_pe"))
        s_dve = ec(nc.semaphore(name="s_dve"))
        s_pb = ec(nc.semaphore(name="s_pb"))
        s_cp = ec(nc.semaphore(name="s_cp"))
        s_out = ec(nc.semaphore(name="s_out"))
        block = ec(nc.Block())

        def toep_ap(c, ri, u):
            jlen = JC[c][1]
            if c < 2:
                base = ((ri * 15 + u) * 2 + c) * 114
                return toep01[:, base : base + jlen]
            base = (ri * 15 + u) * 13
            return toep2[:, base : base + jlen]

        def timg_ap(c, ri, u):
            v = timg[c][:].rearrange("p (a b) -> p a b", a=BC)
            return v[:, :, 16 * ri + u : 16 * ri + u + 16]

        @block.sync
        def _(sync):
            HT = 15 * 2 * 114
            for c in range(3):
                x0 = JC[c][2] if c > 0 else 0
                sync.dma_start(
                    timg[c][:],
                    d_timg[x0 : x0 + XL[c]].rearrange("p a b -> p (a b)"),
                ).then_inc(s_in, 16)
            sync.dma_start(
                toep01[:, :HT],
                d_toep01[:, 0:1].rearrange("p r a b j -> p (r a b j)"),
            ).then_inc(s_in, 16)
            sync.dma_start(toep2[:], d_toep2[:].rearrange("p a b j -> p (a b j)")).then_inc(s_in, 16)
            sync.dma_start(
                toep01[:, HT:],
                d_toep01[:, 1:2].rearrange("p r a b j -> p (r a b j)"),
            ).then_inc(s_in, 16)
            sync.dma_start(mask[:], d_mask[:].rearrange("p a b c -> p (a b c)")).then_inc(s_in, 16)
            sync.dma_start(band01[:], d_band01[:].rearrange("p a b -> p (a b)")).then_inc(s_in, 16)
            sync.dma_start(band2[:], d_band2[:]).then_inc(s_in, 16)
            for m in range(6):
                sync.wait_ge(s_cp, m + 1)
                sync.dma_start(
                    d_out[2 * NY * m : 2 * NY * (m + 1)],
                    outbuf[: 2 * NY, 256 * m : 256 * (m + 1)],
                ).then_inc(s_out, 16)

        @block.tensor
        def _(tensor):
            # ri=0 needs timg(48) + toep01_ri0(64) + toep2(80);
            # ri=1 additionally toep01_ri1(96)
            for ri in range(2):
                tensor.wait_ge(s_in, 80 if ri == 0 else 96)
                for c in range(3):
                    jlen = JC[c][1]
                    g = ri * 3 + c
                    pview = ps[g][:jlen, : BC * 16].rearrange("p (a b) -> p a b", a=BC)
                    for u in range(15):
                        mm = nc.tensor.matmul(
                            pview,
                            toep_ap(c, ri, u),
                            timg_ap(c, ri, u),
                            start=(u == 0),
                            stop=(u == 14),
                        )
                        if u == 14:
                            mm.then_inc(s_pe, 1)
            tensor.wait_ge(s_in, 144)  # band01/band2 landed
            tensor.wait_ge(s_dve, 3)
            for m in range(6):
                if m >= 2:
                    tensor.wait_ge(s_cp, m - 1)
                for c in range(3):
                    jlen = JC[c][1]
                    rhs = (
                        band01[:jlen, 256 * c : 256 * (c + 1)]
                        if c < 2
                        else band2[:jlen, :]
                    )
                    mm = nc.tensor.matmul(
                        pb[m % 2][: 2 * NY, :256],
                        sbf[c][:jlen, 2 * NY * m : 2 * NY * (m + 1)],
                        rhs,
                        start=(c == 0),
                        stop=(c == 2),
                    )
                    if c == 2:
                        mm.then_inc(s_pb, 1)

        @block.vector
        def _(vector):
            for c in range(3):
                nc.vector.memset(corrT[c][:], 0.0)
            vector.wait_ge(s_in, 112)  # mask landed
            for ri in range(2):
                for c in range(3):
                    jlen = JC[c][1]
                    g = ri * 3 + c
                    vector.wait_ge(s_pe, g + 1)
                    dst = corrT[c][:jlen].rearrange("p (a b) -> p a b", a=BC)[
                        :, :, 15 + 16 * ri : 31 + 16 * ri
                    ]
                    mview = mask[:jlen].rearrange("p (r a b) -> p r a b", r=2, a=BC)
                    pview = ps[g][:jlen, : BC * 16].rearrange("p (a b) -> p a b", a=BC)
                    nc.vector.tensor_mul(dst, pview, mview[:, ri])
            for c in range(3):
                jlen = JC[c][1]
                cv = corrT[c][:jlen].rearrange("p (a b) -> p a b", a=BC)
                av = sa[c][:jlen].rearrange("p (a b) -> p a b", a=BC)
                bv = sb[c][:jlen].rearrange("p (a b) -> p a b", a=BC)
                fv = sbf[c][:jlen].rearrange("p (a b) -> p a b", a=BC)
                ln = NI - 1  # 61
                nc.vector.tensor_add(av[:, :, :ln], cv[:, :, :ln], cv[:, :, 1 : 1 + ln])
                ln -= 2  # 59
                nc.vector.tensor_add(bv[:, :, :ln], av[:, :, :ln], av[:, :, 2 : 2 + ln])
                ln -= 4  # 55
                nc.vector.tensor_add(av[:, :, :ln], bv[:, :, :ln], bv[:, :, 4 : 4 + ln])
                ln -= 8  # 47
                nc.vector.tensor_add(
                    fv[:, :, :ln], av[:, :, :ln], av[:, :, 8 : 8 + ln]
                ).then_inc(s_dve, 1)
            for m in range(6):
                vector.wait_ge(s_pb, m + 1)
                nc.vector.tensor_copy(
                    outbuf[: 2 * NY, 256 * m : 256 * (m + 1)],
                    pb[m % 2][: 2 * NY, :256],
                ).then_inc(s_cp, 1)

    return nc


def _build_bass_tile_unused():
    import concourse.bass as bass
    import concourse.mybir as mybir
    from concourse import tile

    f32 = mybir.dt.float32
    bf16 = mybir.dt.bfloat16
    nc = bass.Bass()
    d_timg = nc.dram_tensor("timg", (XP + 1, BC, NR), bf16, kind="ExternalInput")
    d_toep01 = nc.dram_tensor("toep01", (128, 2, 15, 2, 114), bf16, kind="ExternalInput")
    d_toep2 = nc.dram_tensor("toep2", (43, 2, 15, 13), bf16, kind="ExternalInput")
    d_band01 = nc.dram_tensor("band01", (128, 2, 256), f32, kind="ExternalInput")
    d_band2 = nc.dram_tensor("band2", (13, 256), f32, kind="ExternalInput")
    d_mask = nc.dram_tensor("mask", (128, 2, BC, 16), f32, kind="ExternalInput")
    d_out = nc.dram_tensor("out", (BC * NY, 256), f32, kind="ExternalOutput")

    with tile.TileContext(nc) as tc:
        with (
            tc.tile_pool(name="const", bufs=1) as cpool,
            tc.tile_pool(name="work", bufs=1) as wpool,
            tc.tile_pool(name="psum", bufs=2, space=bass.MemorySpace.PSUM) as ppool,
            tc.tile_pool(name="psum_o", bufs=2, space=bass.MemorySpace.PSUM) as opool,
        ):
            # ---- stage in constants ----
            timg = [cpool.tile([XL[c], BC, NR], f32, tag=f"timg{c}", name=f"timg{c}") for c in range(3)]
            for c in range(3):
                x0 = JC[c][2] if c > 0 else 0
                nc.gpsimd.dma_start(timg[c][:], d_timg[x0 : x0 + XL[c]])
            toep01 = cpool.tile([128, 2, 15, 2, 114], f32)
            nc.gpsimd.dma_start(toep01[:], d_toep01[:])
            toep2 = cpool.tile([43, 2, 15, 13], f32)
            nc.gpsimd.dma_start(toep2[:], d_toep2[:])
            band01 = cpool.tile([128, 2, 256], f32)
            nc.gpsimd.dma_start(band01[:], d_band01[:])
            band2 = cpool.tile([13, 256], f32)
            nc.gpsimd.dma_start(band2[:], d_band2[:])
            mask = cpool.tile([128, 2, BC, 16], f32)
            nc.gpsimd.dma_start(mask[:], d_mask[:])

            # corrT[c][jl, bc, i_pad], i_pad = 15 + 16*ri + i
            corrT = [wpool.tile([128, BC, NI], f32, tag=f"corrT{c}", name=f"corrT{c}") for c in range(3)]
            for c in range(3):
                nc.vector.memset(corrT[c][:], 0.0)

            # ---- main matmuls: corr via 15 accumulated taps ----
            for ri in range(2):
                ps = [ppool.tile([128, BC, 16], f32, tag=f"ps{c}", name=f"ps{c}") for c in range(3)]
                for u in range(15):
                    for c in range(3):
                        jlen = JC[c][1]
                        lhsT = (
                            toep01[:, ri, u, c, :jlen]
                            if c < 2
                            else toep2[:, ri, u, :jlen]
                        )
                        rhs = timg[c][:, :, 16 * ri + u : 16 * ri + u + 16]
                        nc.tensor.matmul(
                            ps[c][:jlen],
                            lhsT,
                            rhs,
                            start=(u == 0),
                            stop=(u == 14),
                        )
                for c in range(3):
                    jlen = JC[c][1]
                    nc.vector.tensor_mul(
                        corrT[c][:jlen, :, 15 + 16 * ri : 31 + 16 * ri],
                        ps[c][:jlen],
                        mask[:jlen, ri],
                    )

            # ---- box-sum along i (free dim): shifts 1,2,4,8 ----
            sa = [wpool.tile([128, BC, NI], f32, tag=f"sa{c}", name=f"sa{c}") for c in range(3)]
            sb = [wpool.tile([128, BC, NI], f32, tag=f"sb{c}", name=f"sb{c}") for c in range(3)]
            sbf = [wpool.tile([128, BC * NY], f32, tag=f"sbf{c}", name=f"sbf{c}") for c in range(3)]
            for c in range(3):
                jlen = JC[c][1]
                ln = NI - 1  # 61
                nc.vector.tensor_add(
                    sa[c][:jlen, :, :ln],
                    corrT[c][:jlen, :, :ln],
                    corrT[c][:jlen, :, 1 : 1 + ln],
                )
                ln -= 2  # 59
                nc.vector.tensor_add(
                    sb[c][:jlen, :, :ln],
                    sa[c][:jlen, :, :ln],
                    sa[c][:jlen, :, 2 : 2 + ln],
                )
                ln -= 4  # 55
                nc.vector.tensor_add(
                    sa[c][:jlen, :, :ln],
                    sb[c][:jlen, :, :ln],
                    sb[c][:jlen, :, 4 : 4 + ln],
                )
                ln -= 8  # 47
                # final step writes a compact [j, bc*47] tile so stage-B's
                # stationary operand has a single free dim
                sbv = sbf[c][:jlen].rearrange("p (a b) -> p a b", a=BC)
                nc.vector.tensor_add(
                    sbv[:, :, :ln],
                    sa[c][:jlen, :, :ln],
                    sa[c][:jlen, :, 8 : 8 + ln],
                )

            # ---- box-sum along x via band-ones matmul; out[(bc,y), x] ----
            outbuf = wpool.tile([128, 6, 256], f32)
            for m in range(6):
                ob = opool.tile([128, 256], f32, tag="ob", name="ob")
                for c in range(3):
                    jlen = JC[c][1]
                    lhsT = sbf[c][:jlen, 2 * NY * m : 2 * NY * (m + 1)]
                    rhs = band01[:jlen, c] if c < 2 else band2[:jlen]
                    nc.tensor.matmul(
                        ob[: 2 * NY],
                        lhsT,
                        rhs,
                        start=(c == 0),
                        stop=(c == 2),
                    )
                nc.vector.tensor_copy(outbuf[: 2 * NY, m], ob[: 2 * NY])
            for m in range(6):
                nc.sync.dma_start(
                    d_out[2 * NY * m : 2 * NY * (m + 1)], outbuf[: 2 * NY, m]
                )
    return nc


def kernel(image, kernels):
    image = np.ascontiguousarray(np.asarray(image, np.float32))
    kernels = np.ascontiguousarray(np.asarray(kernels, np.float32))
    from concourse import bass_utils

    if "nc" not in _CACHE:
        _CACHE["nc"] = _build_bass()
    nc = _CACHE["nc"]
    in_maps = _build_core_inputs(image, kernels)
    import time as _time

    trace = bool(int(os.environ.get("KTRACE", "0")))
    try:
        t0 = _time.time()
        res = bass_utils.run_bass_kernel_spmd(
            nc, in_maps, core_ids=list(range(NCORES)), trace=trace
        )
        dt = _time.time() - t0
    except ModuleNotFoundError:
        # axon NTFF profiling hook unavailable in this container
        t0 = _time.time()
        res = bass_utils.run_bass_kernel_spmd(
            nc, in_maps, core_ids=list(range(NCORES)), trace=False
        )
        dt = _time.time() - t0
    if res.exec_time_ns is not None:
        print(f"HW exec time: {res.exec_time_ns} ns")
    else:
        # no profiler available: report end-to-end device dispatch wall
        # (upper bound on HW exec; includes PJRT transfer + launch)
        print(f"HW exec time: {int(dt * 1e9)} ns (wall upper bound, no NTFF hook)")
    out = np.zeros((BC, H, W), np.float32)
    for k in range(NCORES):
        slab = res.results[k]["out"].reshape(BC, NY, 256)
        y0 = 32 * k
        y1 = min(H, y0 + NY)
        out[:, y0:y1, :] += slab[:, : y1 - y0, :]
    return out.reshape(B, C, H, W)


# revision 21
# speedup vs baseline: 1.6656x; 1.2792x over previous
"""Region-wise 15x15 conv + 16x16 box-sum on 8 Trainium2 cores.

Math (per b,c):
  corr[i,j] = sum_{u,v} padded_img[i+u, j+v] * kernels[(i//16)*16 + j//16, u, v]
              for i,j in [0,241)   (padded = 7-zero-pad of 256x256 image)
  out[y,x]  = sum_{i in [y-15,y], j in [x-15,x]} corr[i,j]    (truncated box)

Sharding: core k owns region-rows {2k, 2k+1} -> corr rows i in [32k, 32k+32).
Each core emits a [12bc, 47, 256] output slab (rows y in [32k, 32k+47));
host overlap-adds slabs.

On-core compute:
  corrT[j, bc, i] = sum_u matmul( ToepW_u[x, j], T_img[x, (bc, i+u)] )
    - ToepW (host-built, banded Toeplitz of kernel row u along x) is the
      PE-stationary operand; x-chunks [0,128),[114,242),[228,271) map to
      j-chunks [0,114),[114,228),[228,241); 15 u-taps accumulate in PSUM.
  box-sum along i: log2 shift-adds (1,2,4,8) on DVE over the free dim.
  box-sum along x: matmul with banded-ones rhs -> out[(bc,y), x] in PSUM.
"""

import os
import sys

import numpy as np

sys.path.insert(0, "/opt/trn_rl_repo/concourse")

KS = 15  # kernel size
R = 16  # region size
P = 7  # pad
B, C, H, W = 4, 3, 256, 256
BC = B * C  # 12
NCORES = 8
HO = H - R + 1  # 241 sliding positions per dim
XP = H + 2 * P  # 270 padded extent; x index range [0, 271) with 1 slack
NR = 47  # padded rows per core (i in [0,32) plus u in [0,15))
NI = 62  # padded i-axis: i_pad = i_local + 15, i_local in [-15, 47)
NY = 47  # output rows per core slab
JC = [(0, 114, 0), (114, 114, 114), (228, 13, 228)]  # (j0, jlen, x0) chunks
XL = [128, 128, 43]  # x-chunk partition counts

_CACHE = {}


def _build_core_inputs(image, kernels):
    """Host-side marshaling: per-core transposed image slabs, Toeplitz
    weight tiles, band-ones tiles, and validity masks."""
    img = image.reshape(BC, H, W)
    ker = kernels.reshape(H // R * (W // R), KS, KS)  # [256, 15, 15]
    in_maps = []
    for k in range(NCORES):
        # T_img[x, bc, r]: padded row (32k + r), padded col x ( = img col x-7)
        timg = np.zeros((XP + 1, BC, NR), np.float32)
        r0 = 32 * k - P
        lo = max(0, r0)
        hi = min(H, r0 + NR)
        timg[P : P + W, :, lo - r0 : hi - r0] = img[:, lo:hi, :].transpose(2, 0, 1)

        # Toeplitz weights (partition-first): toep01[xl, ri, u, c, jl] =
        #   ker[kidx, u, v] with v = (x0_c + xl) - (j0_c + jl) in [0,15),
        #   kidx = rg*16 + jg//16
        toep01 = np.zeros((128, 2, 15, 2, 114), np.float32)
        toep2 = np.zeros((43, 2, 15, 13), np.float32)
        for ri in range(2):
            rg = 2 * k + ri
            for c in range(3):
                j0, jlen, x0 = JC[c]
                xl = np.arange(XL[c])[:, None]
                jl = np.arange(jlen)[None, :]
                v = (x0 + xl) - (j0 + jl)
                band = (v >= 0) & (v < KS)
                vv = np.clip(v, 0, KS - 1)
                kk = np.broadcast_to(rg * 16 + (j0 + jl) // 16, v.shape)
                # gather all 15 u-taps at once: [xl, jl, u] -> [xl, u, jl]
                vals = np.where(band[:, :, None], ker[kk, :, vv], 0.0)
                vals = vals.transpose(0, 2, 1)
                if c < 2:
                    toep01[: XL[c], ri, :, c, :jlen] = vals
                else:
                    toep2[: XL[c], ri, :, :jlen] = vals

        # band-ones for the x-direction box sum: band[jl, x] = 1 if 0 <= x - jg <= 15
        band01 = np.zeros((128, 2, 256), np.float32)
        band2 = np.zeros((13, 256), np.float32)
        xs = np.arange(256)[None, :]
        for c in range(3):
            j0, jlen, _ = JC[c]
            jg = j0 + np.arange(jlen)[:, None]
            bb = ((xs - jg >= 0) & (xs - jg <= KS)).astype(np.float32)
            if c < 2:
                band01[:jlen, c, :] = bb
            else:
                band2[:jlen, :] = bb

        # validity mask over (ri, bc, i): i_global = (2k+ri)*16 + i < 241
        mask = np.zeros((128, 2, BC, 16), np.float32)
        for ri in range(2):
            ig = (2 * k + ri) * 16 + np.arange(16)
            mask[:, ri, :, :] = (ig < HO).astype(np.float32)[None, None, :]

        import ml_dtypes

        bf = ml_dtypes.bfloat16
        in_maps.append(
            {
                "timg": timg.astype(bf),
                "toep01": np.ascontiguousarray(toep01).astype(bf),
                "toep2": np.ascontiguousarray(toep2).astype(bf),
                "band01": np.ascontiguousarray(band01),
                "band2": np.ascontiguousarray(band2),
                "mask": np.ascontiguousarray(mask),
            }
        )
    return in_maps


def _build_bass():
    """Raw bass (explicit semaphores, <=1 wait per instruction — this
    walrus build rejects multi-wait instructions)."""
    import contextlib

    import concourse.bass as bass
    import concourse.mybir as mybir

    f32 = mybir.dt.float32
    bf16 = mybir.dt.bfloat16
    nc = bass.Bass()
    d_timg = nc.dram_tensor("timg", (XP + 1, BC, NR), bf16, kind="ExternalInput")
    d_toep01 = nc.dram_tensor("toep01", (128, 2, 15, 2, 114), bf16, kind="ExternalInput")
    d_toep2 = nc.dram_tensor("toep2", (43, 2, 15, 13), bf16, kind="ExternalInput")
    d_band01 = nc.dram_tensor("band01", (128, 2, 256), f32, kind="ExternalInput")
    d_band2 = nc.dram_tensor("band2", (13, 256), f32, kind="ExternalInput")
    d_mask = nc.dram_tensor("mask", (128, 2, BC, 16), f32, kind="ExternalInput")
    d_out = nc.dram_tensor("out", (BC * NY, 256), f32, kind="ExternalOutput")

    with contextlib.ExitStack() as st:
        ec = st.enter_context
        # SBUF
        timg = [ec(nc.sbuf_tensor(f"s_timg{c}", [XL[c], BC * NR], bf16)) for c in range(3)]
        toep01 = ec(nc.sbuf_tensor("s_toep01", [128, 2 * 15 * 2 * 114], bf16))
        toep2 = ec(nc.sbuf_tensor("s_toep2", [43, 2 * 15 * 13], bf16))
        band01 = ec(nc.sbuf_tensor("s_band01", [128, 2 * 256], f32))
        band2 = ec(nc.sbuf_tensor("s_band2", [13, 256], f32))
        mask = ec(nc.sbuf_tensor("s_mask", [128, 2 * BC * 16], f32))
        corrT = [ec(nc.sbuf_tensor(f"s_corrT{c}", [128, BC * NI], f32)) for c in range(3)]
        sa = [ec(nc.sbuf_tensor(f"s_sa{c}", [128, BC * NI], f32)) for c in range(3)]
        sb = [ec(nc.sbuf_tensor(f"s_sb{c}", [128, BC * NI], f32)) for c in range(3)]
        sbf = [ec(nc.sbuf_tensor(f"s_sbf{c}", [128, BC * NY], f32)) for c in range(3)]
        outbuf = ec(nc.sbuf_tensor("s_outbuf", [128, 6 * 256], f32))
        # PSUM: full-bank tensors to guarantee bank exclusivity
        ps = [ec(nc.psum_tensor(f"p_ps{g}", [128, 512], f32)) for g in range(6)]
        pb = [ec(nc.psum_tensor(f"p_pb{g}", [128, 512], f32)) for g in range(2)]
        # semaphores
        s_in = ec(nc.semaphore(name="s_in"))
        s_pe = ec(nc.semaphore(name="s_pe"))
        s_dve = ec(nc.semaphore(name="s_dve"))
        s_pb = ec(nc.semaphore(name="s_pb"))
        s_cp = ec(nc.semaphore(name="s_cp"))
        s_out = ec(nc.semaphore(name="s_out"))
        block = ec(nc.Block())

        def toep_ap(c, ri, u):
            jlen = JC[c][1]
            if c < 2:
                base = ((ri * 15 + u) * 2 + c) * 114
                return toep01[:, base : base + jlen]
            base = (ri * 15 + u) * 13
            return toep2[:, base : base + jlen]

        def timg_ap(c, ri, u):
            v = timg[c][:].rearrange("p (a b) -> p a b", a=BC)
            return v[:, :, 16 * ri + u : 16 * ri + u + 16]

        @block.sync
        def _(sync):
            for c in range(3):
                x0 = JC[c][2] if c > 0 else 0
                sync.dma_start(
                    timg[c][:],
                    d_timg[x0 : x0 + XL[c]].rearrange("p a b -> p (a b)"),
                ).then_inc(s_in, 16)
            sync.dma_start(toep2[:], d_toep2[:].rearrange("p a b j -> p (a b j)")).then_inc(s_in, 16)
            # stream toep01 in 6 chunks of 5 u-taps so the PE can start early
            for ri in range(2):
                for u0 in range(0, 15, 5):
                    sync.dma_start(
                        toep01[:, ((ri * 15 + u0) * 2) * 114 : ((ri * 15 + u0 + 5) * 2) * 114],
                        d_toep01[:, ri : ri + 1, u0 : u0 + 5].rearrange(
                            "p r a b j -> p (r a b j)"
                        ),
                    ).then_inc(s_in, 16)
            sync.dma_start(mask[:], d_mask[:].rearrange("p a b c -> p (a b c)")).then_inc(s_in, 16)
            sync.dma_start(band01[:], d_band01[:].rearrange("p a b -> p (a b)")).then_inc(s_in, 16)
            sync.dma_start(band2[:], d_band2[:]).then_inc(s_in, 16)
            for m in range(6):
                sync.wait_ge(s_cp, m + 1)
                sync.dma_start(
                    d_out[2 * NY * m : 2 * NY * (m + 1)],
                    outbuf[: 2 * NY, 256 * m : 256 * (m + 1)],
                ).then_inc(s_out, 16)

        @block.tensor
        def _(tensor):
            # progressive start: each 5-tap weight chunk unblocks 15 matmuls
            for ri in range(2):
                for u0 in range(0, 15, 5):
                    tensor.wait_ge(s_in, 80 + 16 * (ri * 3 + u0 // 5))
                    for u in range(u0, u0 + 5):
                        for c in range(3):
                            jlen = JC[c][1]
                            g = ri * 3 + c
                            pview = ps[g][:jlen, : BC * 16].rearrange(
                                "p (a b) -> p a b", a=BC
                            )
                            mm = nc.tensor.matmul(
                                pview,
                                toep_ap(c, ri, u),
                                timg_ap(c, ri, u),
                                start=(u == 0),
                                stop=(u == 14),
                            )
                            if u == 14:
                                mm.then_inc(s_pe, 1)
            tensor.wait_ge(s_in, 208)  # band01/band2 landed
            tensor.wait_ge(s_dve, 3)
            for m in range(6):
                if m >= 2:
                    tensor.wait_ge(s_cp, m - 1)
                for c in range(3):
                    jlen = JC[c][1]
                    rhs = (
                        band01[:jlen, 256 * c : 256 * (c + 1)]
                        if c < 2
                        else band2[:jlen, :]
                    )
                    mm = nc.tensor.matmul(
                        pb[m % 2][: 2 * NY, :256],
                        sbf[c][:jlen, 2 * NY * m : 2 * NY * (m + 1)],
                        rhs,
                        start=(c == 0),
                        stop=(c == 2),
                    )
                    if c == 2:
                        mm.then_inc(s_pb, 1)

        @block.vector
        def _(vector):
            for c in range(3):
                nc.vector.memset(corrT[c][:], 0.0)
            vector.wait_ge(s_in, 176)  # mask landed
            for ri in range(2):
                for c in range(3):
                    jlen = JC[c][1]
                    g = ri * 3 + c
                    vector.wait_ge(s_pe, g + 1)
                    dst = corrT[c][:jlen].rearrange("p (a b) -> p a b", a=BC)[
                        :, :, 15 + 16 * ri : 31 + 16 * ri
                    ]
                    mview = mask[:jlen].rearrange("p (r a b) -> p r a b", r=2, a=BC)
                    pview = ps[g][:jlen, : BC * 16].rearrange("p (a b) -> p a b", a=BC)
                    nc.vector.tensor_mul(dst, pview, mview[:, ri])
            for c in range(3):
                jlen = JC[c][1]
                cv = corrT[c][:jlen].rearrange("p (a b) -> p a b", a=BC)
                av = sa[c][:jlen].rearrange("p (a b) -> p a b", a=BC)
                bv = sb[c][:jlen].rearrange("p (a b) -> p a b", a=BC)
                fv = sbf[c][:jlen].rearrange("p (a b) -> p a b", a=BC)
                ln = NI - 1  # 61
                nc.vector.tensor_add(av[:, :, :ln], cv[:, :, :ln], cv[:, :, 1 : 1 + ln])
                ln -= 2  # 59
                nc.vector.tensor_add(bv[:, :, :ln], av[:, :, :ln], av[:, :, 2 : 2 + ln])
                ln -= 4  # 55
                nc.vector.tensor_add(av[:, :, :ln], bv[:, :, :ln], bv[:, :, 4 : 4 + ln])
                ln -= 8  # 47
                nc.vector.tensor_add(
                    fv[:, :, :ln], av[:, :, :ln], av[:, :, 8 : 8 + ln]
                ).then_inc(s_dve, 1)
            for m in range(6):
                vector.wait_ge(s_pb, m + 1)
                nc.vector.tensor_copy(
                    outbuf[: 2 * NY, 256 * m : 256 * (m + 1)],
                    pb[m % 2][: 2 * NY, :256],
                ).then_inc(s_cp, 1)

    return nc


def _build_bass_tile_unused():
    import concourse.bass as bass
    import concourse.mybir as mybir
    from concourse import tile

    f32 = mybir.dt.float32
    bf16 = mybir.dt.bfloat16
    nc = bass.Bass()
    d_timg = nc.dram_tensor("timg", (XP + 1, BC, NR), bf16, kind="ExternalInput")
    d_toep01 = nc.dram_tensor("toep01", (128, 2, 15, 2, 114), bf16, kind="ExternalInput")
    d_toep2 = nc.dram_tensor("toep2", (43, 2, 15, 13), bf16, kind="ExternalInput")
    d_band01 = nc.dram_tensor("band01", (128, 2, 256), f32, kind="ExternalInput")
    d_band2 = nc.dram_tensor("band2", (13, 256), f32, kind="ExternalInput")
    d_mask = nc.dram_tensor("mask", (128, 2, BC, 16), f32, kind="ExternalInput")
    d_out = nc.dram_tensor("out", (BC * NY, 256), f32, kind="ExternalOutput")

    with tile.TileContext(nc) as tc:
        with (
            tc.tile_pool(name="const", bufs=1) as cpool,
            tc.tile_pool(name="work", bufs=1) as wpool,
            tc.tile_pool(name="psum", bufs=2, space=bass.MemorySpace.PSUM) as ppool,
            tc.tile_pool(name="psum_o", bufs=2, space=bass.MemorySpace.PSUM) as opool,
        ):
            # ---- stage in constants ----
            timg = [cpool.tile([XL[c], BC, NR], f32, tag=f"timg{c}", name=f"timg{c}") for c in range(3)]
            for c in range(3):
                x0 = JC[c][2] if c > 0 else 0
                nc.gpsimd.dma_start(timg[c][:], d_timg[x0 : x0 + XL[c]])
            toep01 = cpool.tile([128, 2, 15, 2, 114], f32)
            nc.gpsimd.dma_start(toep01[:], d_toep01[:])
            toep2 = cpool.tile([43, 2, 15, 13], f32)
            nc.gpsimd.dma_start(toep2[:], d_toep2[:])
            band01 = cpool.tile([128, 2, 256], f32)
            nc.gpsimd.dma_start(band01[:], d_band01[:])
            band2 = cpool.tile([13, 256], f32)
            nc.gpsimd.dma_start(band2[:], d_band2[:])
            mask = cpool.tile([128, 2, BC, 16], f32)
            nc.gpsimd.dma_start(mask[:], d_mask[:])

            # corrT[c][jl, bc, i_pad], i_pad = 15 + 16*ri + i
            corrT = [wpool.tile([128, BC, NI], f32, tag=f"corrT{c}", name=f"corrT{c}") for c in range(3)]
            for c in range(3):
                nc.vector.memset(corrT[c][:], 0.0)

            # ---- main matmuls: corr via 15 accumulated taps ----
            for ri in range(2):
                ps = [ppool.tile([128, BC, 16], f32, tag=f"ps{c}", name=f"ps{c}") for c in range(3)]
                for u in range(15):
                    for c in range(3):
                        jlen = JC[c][1]
                        lhsT = (
                            toep01[:, ri, u, c, :jlen]
                            if c < 2
                            else toep2[:, ri, u, :jlen]
                        )
                        rhs = timg[c][:, :, 16 * ri + u : 16 * ri + u + 16]
                        nc.tensor.matmul(
                            ps[c][:jlen],
                            lhsT,
                            rhs,
                            start=(u == 0),
                            stop=(u == 14),
                        )
                for c in range(3):
                    jlen = JC[c][1]
                    nc.vector.tensor_mul(
                        corrT[c][:jlen, :, 15 + 16 * ri : 31 + 16 * ri],
                        ps[c][:jlen],
                        mask[:jlen, ri],
                    )

            # ---- box-sum along i (free dim): shifts 1,2,4,8 ----
            sa = [wpool.tile([128, BC, NI], f32, tag=f"sa{c}", name=f"sa{c}") for c in range(3)]
            sb = [wpool.tile([128, BC, NI], f32, tag=f"sb{c}", name=f"sb{c}") for c in range(3)]
            sbf = [wpool.tile([128, BC * NY], f32, tag=f"sbf{c}", name=f"sbf{c}") for c in range(3)]
            for c in range(3):
                jlen = JC[c][1]
                ln = NI - 1  # 61
                nc.vector.tensor_add(
                    sa[c][:jlen, :, :ln],
                    corrT[c][:jlen, :, :ln],
                    corrT[c][:jlen, :, 1 : 1 + ln],
                )
                ln -= 2  # 59
                nc.vector.tensor_add(
                    sb[c][:jlen, :, :ln],
                    sa[c][:jlen, :, :ln],
                    sa[c][:jlen, :, 2 : 2 + ln],
                )
                ln -= 4  # 55
                nc.vector.tensor_add(
                    sa[c][:jlen, :, :ln],
                    sb[c][:jlen, :, :ln],
                    sb[c][:jlen, :, 4 : 4 + ln],
                )
                ln -= 8  # 47
                # final step writes a compact [j, bc*47] tile so stage-B's
                # stationary operand has a single free dim
                sbv = sbf[c][:jlen].rearrange("p (a b) -> p a b", a=BC)
                nc.vector.tensor_add(
                    sbv[:, :, :ln],
                    sa[c][:jlen, :, :ln],
                    sa[c][:jlen, :, 8 : 8 + ln],
                )

            # ---- box-sum along x via band-ones matmul; out[(bc,y), x] ----
            outbuf = wpool.tile([128, 6, 256], f32)
            for m in range(6):
                ob = opool.tile([128, 256], f32, tag="ob", name="ob")
                for c in range(3):
                    jlen = JC[c][1]
                    lhsT = sbf[c][:jlen, 2 * NY * m : 2 * NY * (m + 1)]
                    rhs = band01[:jlen, c] if c < 2 else band2[:jlen]
                    nc.tensor.matmul(
                        ob[: 2 * NY],
                        lhsT,
                        rhs,
                        start=(c == 0),
                        stop=(c == 2),
                    )
                nc.vector.tensor_copy(outbuf[: 2 * NY, m], ob[: 2 * NY])
            for m in range(6):
                nc.sync.dma_start(
                    d_out[2 * NY * m : 2 * NY * (m + 1)], outbuf[: 2 * NY, m]
                )
    return nc


def _run_spmd_cached(in_maps):
    """Jit-cached clone of bass2jax.run_bass_via_pjrt's multi-core path:
    the shard_map'd _body is traced once and reused, instead of re-tracing
    on every call."""
    import jax
    import concourse.mybir as mybir
    from concourse import bass2jax

    nc = _CACHE["nc"]
    if "runner" not in _CACHE:
        bass2jax.install_neuronx_cc_hook()
        partition_name = (
            nc.partition_id_tensor.name if nc.partition_id_tensor else None
        )
        in_names, out_names, out_avals = [], [], []
        for alloc in nc.m.functions[0].allocations:
            if not isinstance(alloc, mybir.MemoryLocationSet):
                continue
            name = alloc.memorylocations[0].name
            if alloc.kind == "ExternalInput":
                if name != partition_name:
                    in_names.append(name)
            elif alloc.kind == "ExternalOutput":
                out_names.append(name)
                out_avals.append(
                    jax.core.ShapedArray(
                        tuple(alloc.tensor_shape), mybir.dt.np(alloc.dtype)
                    )
                )
        n_params = len(in_names)
        all_names = tuple(in_names + out_names + ([partition_name] if partition_name else []))
        donate = tuple(range(n_params, n_params + len(out_names)))

        def _body(*args):
            operands = list(args)
            if partition_name is not None:
                operands.append(bass2jax.partition_id_tensor())
            return tuple(
                bass2jax._bass_exec_p.bind(
                    *operands,
                    out_avals=tuple(out_avals),
                    in_names=all_names,
                    out_names=tuple(out_names),
                    lowering_input_output_aliases=(),
                    sim_require_finite=True,
                    sim_require_nnan=True,
                    nc=nc,
                )
            )

        devices = jax.devices()[:NCORES]
        mesh = bass2jax.Mesh(np.asarray(devices), ("core",))
        nio = n_params + len(out_names)
        sharded = jax.jit(
            bass2jax.shard_map(
                _body,
                mesh=mesh,
                in_specs=(bass2jax.PartitionSpec("core"),) * nio,
                out_specs=(bass2jax.PartitionSpec("core"),) * len(out_names),
                check_rep=False,
            ),
            donate_argnums=donate,
            keep_unused=True,
        )
        _CACHE["runner"] = (sharded, in_names, out_names, out_avals)
    sharded, in_names, out_names, out_avals = _CACHE["runner"]
    concat_in = [
        np.concatenate([np.asarray(in_maps[c][n]) for c in range(NCORES)], axis=0)
        for n in in_names
    ]
    concat_zeros = [
        np.zeros((NCORES * a.shape[0], *a.shape[1:]), a.dtype) for a in out_avals
    ]
    out_arrs = sharded(*concat_in, *concat_zeros)
    return [
        {
            n: np.asarray(out_arrs[i]).reshape(NCORES, *out_avals[i].shape)[c]
            for i, n in enumerate(out_names)
        }
        for c in range(NCORES)
    ]


def kernel(image, kernels):
    image = np.ascontiguousarray(np.asarray(image, np.float32))
    kernels = np.ascontiguousarray(np.asarray(kernels, np.float32))

    if "nc" not in _CACHE:
        _CACHE["nc"] = _build_bass()
    nc = _CACHE["nc"]
    in_maps = _build_core_inputs(image, kernels)
    import time as _time

    t0 = _time.time()
    try:
        results = _run_spmd_cached(in_maps)
    except Exception:
        _CACHE.pop("runner", None)
        from concourse import bass_utils

        res = bass_utils.run_bass_kernel_spmd(
            nc, in_maps, core_ids=list(range(NCORES)), trace=False
        )
        results = res.results
    dt = _time.time() - t0

    class _R:
        pass

    res = _R()
    res.results = results
    res.exec_time_ns = None
    # no NTFF profiler in this container: report device dispatch wall
    # (upper bound on HW exec; includes PJRT transfer + launch)
    print(f"HW exec time: {int(dt * 1e9)} ns (wall upper bound, no NTFF hook)")
    out = np.zeros((BC, H, W), np.float32)
    for k in range(NCORES):
        slab = res.results[k]["out"].reshape(BC, NY, 256)
        y0 = 32 * k
        y1 = min(H, y0 + NY)
        out[:, y0:y1, :] += slab[:, : y1 - y0, :]
    return out.reshape(B, C, H, W)


# revision 22
# speedup vs baseline: 1.6793x; 1.0082x over previous
"""Region-wise 15x15 conv + 16x16 box-sum on 8 Trainium2 cores.

Math (per b,c):
  corr[i,j] = sum_{u,v} padded_img[i+u, j+v] * kernels[(i//16)*16 + j//16, u, v]
              for i,j in [0,241)   (padded = 7-zero-pad of 256x256 image)
  out[y,x]  = sum_{i in [y-15,y], j in [x-15,x]} corr[i,j]    (truncated box)

Sharding: core k owns region-rows {2k, 2k+1} -> corr rows i in [32k, 32k+32).
Each core emits a [12bc, 47, 256] output slab (rows y in [32k, 32k+47));
host overlap-adds slabs.

On-core compute:
  corrT[j, bc, i] = sum_u matmul( ToepW_u[x, j], T_img[x, (bc, i+u)] )
    - ToepW (host-built, banded Toeplitz of kernel row u along x) is the
      PE-stationary operand; x-chunks [0,128),[114,242),[228,271) map to
      j-chunks [0,114),[114,228),[228,241); 15 u-taps accumulate in PSUM.
  box-sum along i: log2 shift-adds (1,2,4,8) on DVE over the free dim.
  box-sum along x: matmul with banded-ones rhs -> out[(bc,y), x] in PSUM.
"""

import os
import sys

import numpy as np

sys.path.insert(0, "/opt/trn_rl_repo/concourse")

KS = 15  # kernel size
R = 16  # region size
P = 7  # pad
B, C, H, W = 4, 3, 256, 256
BC = B * C  # 12
NCORES = 8
HO = H - R + 1  # 241 sliding positions per dim
XP = H + 2 * P  # 270 padded extent; x index range [0, 271) with 1 slack
NR = 47  # padded rows per core (i in [0,32) plus u in [0,15))
NI = 62  # padded i-axis: i_pad = i_local + 15, i_local in [-15, 47)
NY = 47  # output rows per core slab
JC = [(0, 114, 0), (114, 114, 114), (228, 13, 228)]  # (j0, jlen, x0) chunks
XL = [128, 128, 43]  # x-chunk partition counts

_CACHE = {}


def _build_core_inputs(image, kernels):
    """Host-side marshaling: per-core transposed image slabs, Toeplitz
    weight tiles, band-ones tiles, and validity masks."""
    img = image.reshape(BC, H, W)
    ker = kernels.reshape(H // R * (W // R), KS, KS)  # [256, 15, 15]
    in_maps = []
    for k in range(NCORES):
        # T_img[x, bc, r]: padded row (32k + r), padded col x ( = img col x-7)
        timg = np.zeros((XP + 1, BC, NR), np.float32)
        r0 = 32 * k - P
        lo = max(0, r0)
        hi = min(H, r0 + NR)
        timg[P : P + W, :, lo - r0 : hi - r0] = img[:, lo:hi, :].transpose(2, 0, 1)

        # Toeplitz weights (partition-first): toep01[xl, ri, u, c, jl] =
        #   ker[kidx, u, v] with v = (x0_c + xl) - (j0_c + jl) in [0,15),
        #   kidx = rg*16 + jg//16
        toep01 = np.zeros((128, 2, 15, 2, 114), np.float32)
        toep2 = np.zeros((43, 2, 15, 13), np.float32)
        for ri in range(2):
            rg = 2 * k + ri
            for c in range(3):
                j0, jlen, x0 = JC[c]
                xl = np.arange(XL[c])[:, None]
                jl = np.arange(jlen)[None, :]
                v = (x0 + xl) - (j0 + jl)
                band = (v >= 0) & (v < KS)
                vv = np.clip(v, 0, KS - 1)
                kk = np.broadcast_to(rg * 16 + (j0 + jl) // 16, v.shape)
                # gather all 15 u-taps at once: [xl, jl, u] -> [xl, u, jl]
                vals = np.where(band[:, :, None], ker[kk, :, vv], 0.0)
                vals = vals.transpose(0, 2, 1)
                if c < 2:
                    toep01[: XL[c], ri, :, c, :jlen] = vals
                else:
                    toep2[: XL[c], ri, :, :jlen] = vals

        # band-ones for the x-direction box sum: band[jl, x] = 1 if 0 <= x - jg <= 15
        band01 = np.zeros((128, 2, 256), np.float32)
        band2 = np.zeros((13, 256), np.float32)
        xs = np.arange(256)[None, :]
        for c in range(3):
            j0, jlen, _ = JC[c]
            jg = j0 + np.arange(jlen)[:, None]
            bb = ((xs - jg >= 0) & (xs - jg <= KS)).astype(np.float32)
            if c < 2:
                band01[:jlen, c, :] = bb
            else:
                band2[:jlen, :] = bb

        # validity mask over (ri, bc, i): i_global = (2k+ri)*16 + i < 241
        mask = np.zeros((128, 2, BC, 16), np.float32)
        for ri in range(2):
            ig = (2 * k + ri) * 16 + np.arange(16)
            mask[:, ri, :, :] = (ig < HO).astype(np.float32)[None, None, :]

        import ml_dtypes

        bf = ml_dtypes.bfloat16
        in_maps.append(
            {
                "timg": timg.astype(bf),
                "toep01": np.ascontiguousarray(toep01).astype(bf),
                "toep2": np.ascontiguousarray(toep2).astype(bf),
                "band01": np.ascontiguousarray(band01),
                "band2": np.ascontiguousarray(band2),
                "mask": np.ascontiguousarray(mask),
            }
        )
    return in_maps


def _build_bass():
    """Raw bass (explicit semaphores, <=1 wait per instruction — this
    walrus build rejects multi-wait instructions)."""
    import contextlib

    import concourse.bass as bass
    import concourse.mybir as mybir

    f32 = mybir.dt.float32
    bf16 = mybir.dt.bfloat16
    nc = bass.Bass()
    d_timg = nc.dram_tensor("timg", (XP + 1, BC, NR), bf16, kind="ExternalInput")
    d_toep01 = nc.dram_tensor("toep01", (128, 2, 15, 2, 114), bf16, kind="ExternalInput")
    d_toep2 = nc.dram_tensor("toep2", (43, 2, 15, 13), bf16, kind="ExternalInput")
    d_band01 = nc.dram_tensor("band01", (128, 2, 256), f32, kind="ExternalInput")
    d_band2 = nc.dram_tensor("band2", (13, 256), f32, kind="ExternalInput")
    d_mask = nc.dram_tensor("mask", (128, 2, BC, 16), f32, kind="ExternalInput")
    d_out = nc.dram_tensor("out", (BC * NY, 256), f32, kind="ExternalOutput")

    with contextlib.ExitStack() as st:
        ec = st.enter_context
        # SBUF
        timg = [ec(nc.sbuf_tensor(f"s_timg{c}", [XL[c], BC * NR], bf16)) for c in range(3)]
        toep01 = ec(nc.sbuf_tensor("s_toep01", [128, 2 * 15 * 2 * 114], bf16))
        toep2 = ec(nc.sbuf_tensor("s_toep2", [43, 2 * 15 * 13], bf16))
        band01 = ec(nc.sbuf_tensor("s_band01", [128, 2 * 256], f32))
        band2 = ec(nc.sbuf_tensor("s_band2", [13, 256], f32))
        mask = ec(nc.sbuf_tensor("s_mask", [128, 2 * BC * 16], f32))
        corrT = [ec(nc.sbuf_tensor(f"s_corrT{c}", [128, BC * NI], f32)) for c in range(3)]
        sa = [ec(nc.sbuf_tensor(f"s_sa{c}", [128, BC * NI], f32)) for c in range(3)]
        sb = [ec(nc.sbuf_tensor(f"s_sb{c}", [128, BC * NI], f32)) for c in range(3)]
        sbf = [ec(nc.sbuf_tensor(f"s_sbf{c}", [128, BC * NY], f32)) for c in range(3)]
        outbuf = ec(nc.sbuf_tensor("s_outbuf", [128, 6 * 256], f32))
        # PSUM: full-bank tensors to guarantee bank exclusivity
        ps = [ec(nc.psum_tensor(f"p_ps{g}", [128, 512], f32)) for g in range(6)]
        pb = [ec(nc.psum_tensor(f"p_pb{g}", [128, 512], f32)) for g in range(2)]
        # semaphores
        s_in = ec(nc.semaphore(name="s_in"))
        s_pe = ec(nc.semaphore(name="s_pe"))
        s_dve = ec(nc.semaphore(name="s_dve"))
        s_pb = ec(nc.semaphore(name="s_pb"))
        s_cp = ec(nc.semaphore(name="s_cp"))
        s_out = ec(nc.semaphore(name="s_out"))
        block = ec(nc.Block())

        def toep_ap(c, ri, u):
            jlen = JC[c][1]
            if c < 2:
                base = ((ri * 15 + u) * 2 + c) * 114
                return toep01[:, base : base + jlen]
            base = (ri * 15 + u) * 13
            return toep2[:, base : base + jlen]

        def timg_ap(c, ri, u):
            v = timg[c][:].rearrange("p (a b) -> p a b", a=BC)
            return v[:, :, 16 * ri + u : 16 * ri + u + 16]

        @block.sync
        def _(sync):
            for c in range(3):
                x0 = JC[c][2] if c > 0 else 0
                sync.dma_start(
                    timg[c][:],
                    d_timg[x0 : x0 + XL[c]].rearrange("p a b -> p (a b)"),
                ).then_inc(s_in, 16)
            sync.dma_start(toep2[:], d_toep2[:].rearrange("p a b j -> p (a b j)")).then_inc(s_in, 16)
            # stream toep01 in 6 chunks of 5 u-taps so the PE can start early
            for ri in range(2):
                for u0 in range(0, 15, 5):
                    sync.dma_start(
                        toep01[:, ((ri * 15 + u0) * 2) * 114 : ((ri * 15 + u0 + 5) * 2) * 114],
                        d_toep01[:, ri : ri + 1, u0 : u0 + 5].rearrange(
                            "p r a b j -> p (r a b j)"
                        ),
                    ).then_inc(s_in, 16)
            sync.dma_start(mask[:], d_mask[:].rearrange("p a b c -> p (a b c)")).then_inc(s_in, 16)
            sync.dma_start(band01[:], d_band01[:].rearrange("p a b -> p (a b)")).then_inc(s_in, 16)
            sync.dma_start(band2[:], d_band2[:]).then_inc(s_in, 16)
            for m in range(6):
                sync.wait_ge(s_cp, m + 1)
                sync.dma_start(
                    d_out[2 * NY * m : 2 * NY * (m + 1)],
                    outbuf[: 2 * NY, 256 * m : 256 * (m + 1)],
                ).then_inc(s_out, 16)

        @block.tensor
        def _(tensor):
            # progressive start: each 5-tap weight chunk unblocks 15 matmuls
            for ri in range(2):
                for u0 in range(0, 15, 5):
                    tensor.wait_ge(s_in, 80 + 16 * (ri * 3 + u0 // 5))
                    for u in range(u0, u0 + 5):
                        for c in range(3):
                            jlen = JC[c][1]
                            g = ri * 3 + c
                            pview = ps[g][:jlen, : BC * 16].rearrange(
                                "p (a b) -> p a b", a=BC
                            )
                            mm = nc.tensor.matmul(
                                pview,
                                toep_ap(c, ri, u),
                                timg_ap(c, ri, u),
                                start=(u == 0),
                                stop=(u == 14),
                            )
                            if u == 14:
                                mm.then_inc(s_pe, 1)
            tensor.wait_ge(s_in, 208)  # band01/band2 landed
            tensor.wait_ge(s_dve, 3)
            for m in range(6):
                if m >= 2:
                    tensor.wait_ge(s_cp, m - 1)
                for c in range(3):
                    jlen = JC[c][1]
                    rhs = (
                        band01[:jlen, 256 * c : 256 * (c + 1)]
                        if c < 2
                        else band2[:jlen, :]
                    )
                    mm = nc.tensor.matmul(
                        pb[m % 2][: 2 * NY, :256],
                        sbf[c][:jlen, 2 * NY * m : 2 * NY * (m + 1)],
                        rhs,
                        start=(c == 0),
                        stop=(c == 2),
                    )
                    if c == 2:
                        mm.then_inc(s_pb, 1)

        @block.vector
        def _(vector):
            for c in range(3):
                nc.vector.memset(corrT[c][:], 0.0)
            vector.wait_ge(s_in, 176)  # mask landed
            for ri in range(2):
                for c in range(3):
                    jlen = JC[c][1]
                    g = ri * 3 + c
                    vector.wait_ge(s_pe, g + 1)
                    dst = corrT[c][:jlen].rearrange("p (a b) -> p a b", a=BC)[
                        :, :, 15 + 16 * ri : 31 + 16 * ri
                    ]
                    mview = mask[:jlen].rearrange("p (r a b) -> p r a b", r=2, a=BC)
                    pview = ps[g][:jlen, : BC * 16].rearrange("p (a b) -> p a b", a=BC)
                    nc.vector.tensor_mul(dst, pview, mview[:, ri])
            for c in range(3):
                jlen = JC[c][1]
                cv = corrT[c][:jlen].rearrange("p (a b) -> p a b", a=BC)
                av = sa[c][:jlen].rearrange("p (a b) -> p a b", a=BC)
                bv = sb[c][:jlen].rearrange("p (a b) -> p a b", a=BC)
                fv = sbf[c][:jlen].rearrange("p (a b) -> p a b", a=BC)
                ln = NI - 1  # 61
                nc.vector.tensor_add(av[:, :, :ln], cv[:, :, :ln], cv[:, :, 1 : 1 + ln])
                ln -= 2  # 59
                nc.vector.tensor_add(bv[:, :, :ln], av[:, :, :ln], av[:, :, 2 : 2 + ln])
                ln -= 4  # 55
                nc.vector.tensor_add(av[:, :, :ln], bv[:, :, :ln], bv[:, :, 4 : 4 + ln])
                ln -= 8  # 47
                nc.vector.tensor_add(
                    fv[:, :, :ln], av[:, :, :ln], av[:, :, 8 : 8 + ln]
                ).then_inc(s_dve, 1)
            for m in range(6):
                vector.wait_ge(s_pb, m + 1)
                nc.vector.tensor_copy(
                    outbuf[: 2 * NY, 256 * m : 256 * (m + 1)],
                    pb[m % 2][: 2 * NY, :256],
                ).then_inc(s_cp, 1)

    return nc


def _build_bass_tile_unused():
    import concourse.bass as bass
    import concourse.mybir as mybir
    from concourse import tile

    f32 = mybir.dt.float32
    bf16 = mybir.dt.bfloat16
    nc = bass.Bass()
    d_timg = nc.dram_tensor("timg", (XP + 1, BC, NR), bf16, kind="ExternalInput")
    d_toep01 = nc.dram_tensor("toep01", (128, 2, 15, 2, 114), bf16, kind="ExternalInput")
    d_toep2 = nc.dram_tensor("toep2", (43, 2, 15, 13), bf16, kind="ExternalInput")
    d_band01 = nc.dram_tensor("band01", (128, 2, 256), f32, kind="ExternalInput")
    d_band2 = nc.dram_tensor("band2", (13, 256), f32, kind="ExternalInput")
    d_mask = nc.dram_tensor("mask", (128, 2, BC, 16), f32, kind="ExternalInput")
    d_out = nc.dram_tensor("out", (BC * NY, 256), f32, kind="ExternalOutput")

    with tile.TileContext(nc) as tc:
        with (
            tc.tile_pool(name="const", bufs=1) as cpool,
            tc.tile_pool(name="work", bufs=1) as wpool,
            tc.tile_pool(name="psum", bufs=2, space=bass.MemorySpace.PSUM) as ppool,
            tc.tile_pool(name="psum_o", bufs=2, space=bass.MemorySpace.PSUM) as opool,
        ):
            # ---- stage in constants ----
            timg = [cpool.tile([XL[c], BC, NR], f32, tag=f"timg{c}", name=f"timg{c}") for c in range(3)]
            for c in range(3):
                x0 = JC[c][2] if c > 0 else 0
                nc.gpsimd.dma_start(timg[c][:], d_timg[x0 : x0 + XL[c]])
            toep01 = cpool.tile([128, 2, 15, 2, 114], f32)
            nc.gpsimd.dma_start(toep01[:], d_toep01[:])
            toep2 = cpool.tile([43, 2, 15, 13], f32)
            nc.gpsimd.dma_start(toep2[:], d_toep2[:])
            band01 = cpool.tile([128, 2, 256], f32)
            nc.gpsimd.dma_start(band01[:], d_band01[:])
            band2 = cpool.tile([13, 256], f32)
            nc.gpsimd.dma_start(band2[:], d_band2[:])
            mask = cpool.tile([128, 2, BC, 16], f32)
            nc.gpsimd.dma_start(mask[:], d_mask[:])

            # corrT[c][jl, bc, i_pad], i_pad = 15 + 16*ri + i
            corrT = [wpool.tile([128, BC, NI], f32, tag=f"corrT{c}", name=f"corrT{c}") for c in range(3)]
            for c in range(3):
                nc.vector.memset(corrT[c][:], 0.0)

            # ---- main matmuls: corr via 15 accumulated taps ----
            for ri in range(2):
                ps = [ppool.tile([128, BC, 16], f32, tag=f"ps{c}", name=f"ps{c}") for c in range(3)]
                for u in range(15):
                    for c in range(3):
                        jlen = JC[c][1]
                        lhsT = (
                            toep01[:, ri, u, c, :jlen]
                            if c < 2
                            else toep2[:, ri, u, :jlen]
                        )
                        rhs = timg[c][:, :, 16 * ri + u : 16 * ri + u + 16]
                        nc.tensor.matmul(
                            ps[c][:jlen],
                            lhsT,
                            rhs,
                            start=(u == 0),
                            stop=(u == 14),
                        )
                for c in range(3):
                    jlen = JC[c][1]
                    nc.vector.tensor_mul(
                        corrT[c][:jlen, :, 15 + 16 * ri : 31 + 16 * ri],
                        ps[c][:jlen],
                        mask[:jlen, ri],
                    )

            # ---- box-sum along i (free dim): shifts 1,2,4,8 ----
            sa = [wpool.tile([128, BC, NI], f32, tag=f"sa{c}", name=f"sa{c}") for c in range(3)]
            sb = [wpool.tile([128, BC, NI], f32, tag=f"sb{c}", name=f"sb{c}") for c in range(3)]
            sbf = [wpool.tile([128, BC * NY], f32, tag=f"sbf{c}", name=f"sbf{c}") for c in range(3)]
            for c in range(3):
                jlen = JC[c][1]
                ln = NI - 1  # 61
                nc.vector.tensor_add(
                    sa[c][:jlen, :, :ln],
                    corrT[c][:jlen, :, :ln],
                    corrT[c][:jlen, :, 1 : 1 + ln],
                )
                ln -= 2  # 59
                nc.vector.tensor_add(
                    sb[c][:jlen, :, :ln],
                    sa[c][:jlen, :, :ln],
                    sa[c][:jlen, :, 2 : 2 + ln],
                )
                ln -= 4  # 55
                nc.vector.tensor_add(
                    sa[c][:jlen, :, :ln],
                    sb[c][:jlen, :, :ln],
                    sb[c][:jlen, :, 4 : 4 + ln],
                )
                ln -= 8  # 47
                # final step writes a compact [j, bc*47] tile so stage-B's
                # stationary operand has a single free dim
                sbv = sbf[c][:jlen].rearrange("p (a b) -> p a b", a=BC)
                nc.vector.tensor_add(
                    sbv[:, :, :ln],
                    sa[c][:jlen, :, :ln],
                    sa[c][:jlen, :, 8 : 8 + ln],
                )

            # ---- box-sum along x via band-ones matmul; out[(bc,y), x] ----
            outbuf = wpool.tile([128, 6, 256], f32)
            for m in range(6):
                ob = opool.tile([128, 256], f32, tag="ob", name="ob")
                for c in range(3):
                    jlen = JC[c][1]
                    lhsT = sbf[c][:jlen, 2 * NY * m : 2 * NY * (m + 1)]
                    rhs = band01[:jlen, c] if c < 2 else band2[:jlen]
                    nc.tensor.matmul(
                        ob[: 2 * NY],
                        lhsT,
                        rhs,
                        start=(c == 0),
                        stop=(c == 2),
                    )
                nc.vector.tensor_copy(outbuf[: 2 * NY, m], ob[: 2 * NY])
            for m in range(6):
                nc.sync.dma_start(
                    d_out[2 * NY * m : 2 * NY * (m + 1)], outbuf[: 2 * NY, m]
                )
    return nc


def _run_spmd_cached(in_maps):
    """Jit-cached clone of bass2jax.run_bass_via_pjrt's multi-core path:
    the shard_map'd _body is traced once and reused, instead of re-tracing
    on every call."""
    import jax
    import concourse.mybir as mybir
    from concourse import bass2jax

    nc = _CACHE["nc"]
    if "runner" not in _CACHE:
        bass2jax.install_neuronx_cc_hook()
        partition_name = (
            nc.partition_id_tensor.name if nc.partition_id_tensor else None
        )
        in_names, out_names, out_avals = [], [], []
        for alloc in nc.m.functions[0].allocations:
            if not isinstance(alloc, mybir.MemoryLocationSet):
                continue
            name = alloc.memorylocations[0].name
            if alloc.kind == "ExternalInput":
                if name != partition_name:
                    in_names.append(name)
            elif alloc.kind == "ExternalOutput":
                out_names.append(name)
                out_avals.append(
                    jax.core.ShapedArray(
                        tuple(alloc.tensor_shape), mybir.dt.np(alloc.dtype)
                    )
                )
        n_params = len(in_names)
        all_names = tuple(in_names + out_names + ([partition_name] if partition_name else []))
        donate = tuple(range(n_params, n_params + len(out_names)))

        def _body(*args):
            operands = list(args)
            if partition_name is not None:
                operands.append(bass2jax.partition_id_tensor())
            return tuple(
                bass2jax._bass_exec_p.bind(
                    *operands,
                    out_avals=tuple(out_avals),
                    in_names=all_names,
                    out_names=tuple(out_names),
                    lowering_input_output_aliases=(),
                    sim_require_finite=True,
                    sim_require_nnan=True,
                    nc=nc,
                )
            )

        devices = jax.devices()[:NCORES]
        mesh = bass2jax.Mesh(np.asarray(devices), ("core",))
        nio = n_params + len(out_names)
        sharded = jax.jit(
            bass2jax.shard_map(
                _body,
                mesh=mesh,
                in_specs=(bass2jax.PartitionSpec("core"),) * nio,
                out_specs=(bass2jax.PartitionSpec("core"),) * len(out_names),
                check_rep=False,
            ),
            donate_argnums=donate,
            keep_unused=True,
        )
        _CACHE["runner"] = (sharded, in_names, out_names, out_avals)
    sharded, in_names, out_names, out_avals = _CACHE["runner"]
    if "bufs" not in _CACHE:
        _CACHE["bufs"] = (
            [
                np.empty(
                    (NCORES * in_maps[0][n].shape[0], *in_maps[0][n].shape[1:]),
                    in_maps[0][n].dtype,
                )
                for n in in_names
            ],
            [np.zeros((NCORES * a.shape[0], *a.shape[1:]), a.dtype) for a in out_avals],
        )
    concat_in, concat_zeros = _CACHE["bufs"]
    for i, n in enumerate(in_names):
        d0 = in_maps[0][n].shape[0]
        for c in range(NCORES):
            concat_in[i][c * d0 : (c + 1) * d0] = in_maps[c][n]
    out_arrs = sharded(*concat_in, *concat_zeros)
    return [
        {
            n: np.asarray(out_arrs[i]).reshape(NCORES, *out_avals[i].shape)[c]
            for i, n in enumerate(out_names)
        }
        for c in range(NCORES)
    ]


def kernel(image, kernels):
    image = np.ascontiguousarray(np.asarray(image, np.float32))
    kernels = np.ascontiguousarray(np.asarray(kernels, np.float32))

    if "nc" not in _CACHE:
        _CACHE["nc"] = _build_bass()
    nc = _CACHE["nc"]
    in_maps = _build_core_inputs(image, kernels)
    import time as _time

    t0 = _time.time()
    try:
        results = _run_spmd_cached(in_maps)
    except Exception:
        _CACHE.pop("runner", None)
        from concourse import bass_utils

        res = bass_utils.run_bass_kernel_spmd(
            nc, in_maps, core_ids=list(range(NCORES)), trace=False
        )
        results = res.results
    dt = _time.time() - t0

    class _R:
        pass

    res = _R()
    res.results = results
    res.exec_time_ns = None
    # no NTFF profiler in this container: report device dispatch wall
    # (upper bound on HW exec; includes PJRT transfer + launch)
    print(f"HW exec time: {int(dt * 1e9)} ns (wall upper bound, no NTFF hook)")
    out = np.zeros((BC, H, W), np.float32)
    for k in range(NCORES):
        slab = res.results[k]["out"].reshape(BC, NY, 256)
        y0 = 32 * k
        y1 = min(H, y0 + NY)
        out[:, y0:y1, :] += slab[:, : y1 - y0, :]
    return out.reshape(B, C, H, W)
